# revision 7
# baseline (speedup 1.0000x reference)
"""Trainium kernel for nn_FCFE_39505109188695.

Self-contained: takes FULL unsharded inputs, runs the fused
conv/FFT/attention model across the 8 NeuronCores (channel-sharded per
the tensor-parallel hint), returns the FULL output.

All FFT/iFFT stages are lowered to real-valued DFT matmuls (cos/sin
basis) so everything maps onto the TensorEngine — the Neuron stack has
no native complex FFT. LayerNorm reductions, the cross-batch attention
softmax and the 1x1-conv channel contractions are left to GSPMD, which
inserts the small all-reduces (channel contractions) while the
depthwise convs and DFT matmuls stay fully channel-parallel.
"""
import numpy as np
import jax
import jax.numpy as jnp
from jax.sharding import Mesh, NamedSharding, PartitionSpec as P

C, B, H, W = 256, 4, 128, 128
EPS = 1e-5

# DFT bases in float64, cast once to f32: A=cos, S=sin for k*n*2pi/N.
_k = np.arange(H, dtype=np.float64)
_ang = 2.0 * np.pi * np.outer(_k, _k) / H
_A = np.cos(_ang).astype(np.float32)
_S = np.sin(_ang).astype(np.float32)


def _ln(x):
    m = x.mean(axis=(1, 2, 3), keepdims=True)
    v = x.var(axis=(1, 2, 3), keepdims=True)
    return (x - m) * jax.lax.rsqrt(v + EPS)


def _c1(x, w, b):
    return jnp.einsum('bchw,oc->bohw', x, w) + b[None, :, None, None]


def _dw(x, w, b):
    # depthwise 3x3 SAME as 9 shifted multiply-adds (pure VectorE work)
    xp = jnp.pad(x, ((0, 0), (0, 0), (1, 1), (1, 1)))
    y = b[None, :, None, None]
    for i in range(3):
        for j in range(3):
            y = y + xp[:, :, i:i + H, j:j + W] * w[:, 0, i, j][None, :, None, None]
    return y


def _mm_last(y, M):   # contract width axis
    return jnp.einsum('bchw,wv->bchv', y, M)


def _mm_pen(M, y):    # contract height axis
    return jnp.einsum('uh,bchv->bcuv', M, y)


def _fft2_parts(y, A, S):
    yA = _mm_last(y, A)
    yS = _mm_last(y, S)
    xr = _mm_pen(A, yA) - _mm_pen(S, yS)
    xi = -(_mm_pen(A, yS) + _mm_pen(S, yA))
    return xr, xi


def _ifft2_real(z, A, S):
    zA = _mm_last(z, A)
    zS = _mm_last(z, S)
    return (_mm_pen(A, zA) - _mm_pen(S, zS)) * (1.0 / (H * W))


def _forward(event_features, image_features, w_c1, b_c1, w_dw1, b_dw1, w_dw2, b_dw2,
             w_dw3, b_dw3, w_re, b_re, w_im, b_im, w_c2, b_c2, w_c3, b_c3,
             w_c4, b_c4, w_c5, b_c5, w_c6, b_c6, w_c7, b_c7, w_ca, b_ca,
             w_fg, b_fg, w_q, b_q, w_k, b_k, w_v, b_v, w_o, b_o, A, S):
    x1 = _ln(event_features)
    x2 = _ln(image_features)
    x3 = _c1(jnp.concatenate([x1, x2], axis=1), w_c1, b_c1)

    x4r, x4i = _fft2_parts(_dw(x3, w_dw1, b_dw1), A, S)
    real = jax.nn.relu(_c1(x4r, w_re, b_re))
    imag = jax.nn.relu(_c1(x4i, w_im, b_im))
    x5 = jax.nn.sigmoid(_c1(jnp.concatenate([real, imag], axis=1), w_c5, b_c5))

    x6r, _x6i = _fft2_parts(_dw(_c1(x2, w_c2, b_c2), w_dw2, b_dw2), A, S)
    z = _c1(jax.nn.relu(_c1(x5 * x6r, w_c4, b_c4)), w_c3, b_c3)
    x7 = _ifft2_real(z, A, S)

    x8r = _mm_last(x7, A)
    x8i = -_mm_last(x7, S)
    x9 = _c1(_c1(x7 + x3, w_ca, b_ca), w_fg, b_fg)
    x10 = (_mm_last(x8r * x9, A) - _mm_last(x8i * x9, S)) * (1.0 / W)

    q = x10.transpose(0, 2, 3, 1).reshape(B, H * W, C)
    kv = _c1(x1, w_c6, b_c6).transpose(0, 2, 3, 1).reshape(B, H * W, C)
    Q = jnp.einsum('lne,fe->lnf', q, w_q) + b_q
    K = jnp.einsum('lne,fe->lnf', kv, w_k) + b_k
    V = jnp.einsum('lne,fe->lnf', kv, w_v) + b_v
    scores = jnp.einsum('lne,mne->nlm', Q, K) * (1.0 / np.float32(np.sqrt(C)))
    Attn = jax.nn.softmax(scores, axis=-1)
    out = jnp.einsum('nlm,mne->lne', Attn, V)
    xc = jnp.einsum('lne,fe->lnf', out, w_o) + b_o
    xc = xc.reshape(B, H, W, C).transpose(0, 3, 1, 2)

    x11 = _c1(jax.nn.gelu(_dw(xc, w_dw3, b_dw3), approximate=False), w_c7, b_c7)
    return x11 + x2


_COMPILED = None

_ORDER = ['event_features', 'image_features', 'w_c1', 'b_c1', 'w_dw1',
          'b_dw1', 'w_dw2', 'b_dw2', 'w_dw3', 'b_dw3', 'w_re', 'b_re',
          'w_im', 'b_im', 'w_c2', 'b_c2', 'w_c3', 'b_c3', 'w_c4', 'b_c4',
          'w_c5', 'b_c5', 'w_c6', 'b_c6', 'w_c7', 'b_c7', 'w_ca', 'b_ca',
          'w_fg', 'b_fg', 'w_q', 'b_q', 'w_k', 'b_k', 'w_v', 'b_v',
          'w_o', 'b_o']


def _run_mode(mode, args):
    """Compile (cached) + run under one backend mode; raises on failure."""
    global _COMPILED
    if _COMPILED is not None and _COMPILED[0] == mode:
        _, fn, dev, _ = _COMPILED
    else:
        fn = dev = None
    if mode == 'shard8':
        if fn is None:
            devs = jax.devices()
            if len(devs) < 8:
                raise RuntimeError('need 8 cores')
            mesh = Mesh(np.array(devs[:8]), ('x',))
            shard_c = NamedSharding(mesh, P(None, 'x'))
            repl = NamedSharding(mesh, P())
            in_sh = tuple(shard_c if n in ('event_features', 'image_features')
                          else repl for n in _ORDER) + (repl, repl)
            fn = jax.jit(_forward, in_shardings=in_sh, out_shardings=shard_c)
        out = np.asarray(jax.device_get(fn(*args)), np.float32)
        _COMPILED = (mode, fn, None, None)
        return out
    if mode == 'single':
        if fn is None:
            dev = jax.devices()[0]
            fn = jax.jit(_forward)
        pargs = [jax.device_put(a, dev) for a in args]
        out = np.asarray(jax.device_get(fn(*pargs)), np.float32)
        _COMPILED = (mode, fn, dev, None)
        return out
    # cpu
    if fn is None:
        dev = jax.devices('cpu')[0]
        fn = jax.jit(_forward)
    pargs = [jax.device_put(a, dev) for a in args]
    out = np.asarray(jax.device_get(fn(*pargs)), np.float32)
    _COMPILED = ('cpu', fn, dev, None)
    return out


def kernel(**inputs):
    args = [np.asarray(inputs[n], np.float32) for n in _ORDER] + [_A, _S]
    start = 'shard8' if _COMPILED is None else _COMPILED[0]
    modes = ['shard8', 'single', 'cpu']
    last_err = None
    for mode in modes[modes.index(start):]:
        try:
            return _run_mode(mode, args)
        except Exception as e:  # fall through to next backend
            last_err = e
    raise RuntimeError(f'all backends failed: {last_err}')
